# revision 1
# baseline (speedup 1.0000x reference)
"""BTT layer (nn_BTTLayer_36885179138559) as a Trainium2 Bass kernel.

Factorized BTT evaluation (no dense-W collapse): per core (data-parallel over
512 of the 4096 flattened batch rows),
  stage 1: inner[n, B, m*8+r] = x_n[B, 64] @ btt_r[n]        (64 matmuls, K=64)
  butterfly: T[(n,r), B] per (m, n-group) via PE transpose; the BTT block
             permutation happens in the stage-1 eviction scatter (S is
             m-major) so each transpose reads 128 contiguous columns
  stage 2: out[B, m*64+a] = sum_g T_slab.T @ btt_l[m, g-slab] (PSUM accum)
4x fewer FLOPs than the dense-W kernel; weights traffic 8MB vs 32MB.
Compute in bf16 (fp32 PSUM), device output bf16, host casts to fp32.
"""

import numpy as np
import ml_dtypes

import concourse.bacc as bacc
import concourse.mybir as mybir
import concourse.tile as tile
import concourse.bass_utils as bass_utils

# problem dims (hardcoded per contract)
M, N, A, B_BLK, RANK = 64, 64, 64, 64, 8
D = 4096              # in = out features
ROWS = 4096           # flattened batch (4, 1024, 4096)
N_CORES = 8
BS = ROWS // N_CORES  # 512 rows per core
BT = 4                # batch tiles of 128

BF16 = mybir.dt.bfloat16
F32 = mybir.dt.float32

_compiled = None
_last_in_maps = None


def _build():
    nc = bacc.Bacc("TRN2", target_bir_lowering=False, debug=False, num_devices=N_CORES)
    # xt/rt: [128, 32, 512]; tile t holds feature rows 128t..128t+128
    # (two 64-row n-blocks per tile), columns = batch rows (xt) / m*8+r (rt)
    xt_ap = nc.dram_tensor("xt", [128, 32, 512], BF16, kind="ExternalInput").ap()
    rt_ap = nc.dram_tensor("rt", [128, 32, 512], BF16, kind="ExternalInput").ap()
    # lt: [128, m, g, a] = btt_l[m, (16g + p//8)*8 + p%8, a]
    lt_ap = nc.dram_tensor("lt", [128, M, 4, A], BF16, kind="ExternalInput").ap()
    id_ap = nc.dram_tensor("ident", [128, 128], BF16, kind="ExternalInput").ap()
    # o: [bt, 128, m*64+a] bf16
    o_ap = nc.dram_tensor("o", [BT, 128, D], BF16, kind="ExternalOutput").ap()

    with tile.TileContext(nc) as tc:
        with (
            tc.tile_pool(name="xin", bufs=1) as xin,
            tc.tile_pool(name="spool", bufs=1) as spool,
            tc.tile_pool(name="tpool", bufs=4) as tpool,
            tc.tile_pool(name="opool", bufs=2) as opool,
            tc.tile_pool(name="ps1p", bufs=4, space="PSUM") as ps1p,
            tc.tile_pool(name="psTp", bufs=2, space="PSUM") as psTp,
            tc.tile_pool(name="ps2p", bufs=2, space="PSUM") as ps2p,
        ):
            # chunked input tiles so the first matmuls start early
            ID = xin.tile([128, 128], BF16, tag="id", name="ID")
            nc.sync.dma_start(ID[:], id_ap)
            XTc, RTc = [], []
            for q in range(4):
                xq = xin.tile([128, 8, 512], BF16, tag=f"xt{q}", name=f"XT{q}")
                rq = xin.tile([128, 8, 512], BF16, tag=f"rt{q}", name=f"RT{q}")
                nc.sync.dma_start(xq[:], xt_ap[:, 8 * q:8 * (q + 1), :])
                nc.sync.dma_start(rq[:], rt_ap[:, 8 * q:8 * (q + 1), :])
                XTc.append(xq)
                RTc.append(rq)
                if q == 1:
                    LT = xin.tile([128, M, 4, A], BF16, tag="lt", name="LT")
                    nc.sync.dma_start(LT[:], lt_ap)

            # inner for one batch tile: S[p=row, m, n*8+r] — m-major free
            # layout so each butterfly transpose reads 128 contiguous cols
            S = spool.tile([128, M, 512], BF16, tag="S", name="S")

            for bt in range(BT):
                # ---- stage 1: one K=64 matmul + one evict per n-block; the
                # evict scatters (m, r) into S's m-major layout. Alternate
                # Act/DVE so eviction keeps pace with the PE.
                for n in range(N):
                    ps1 = ps1p.tile([128, 512], F32, tag="ps1", name=f"ps1_{bt}_{n}")
                    pl, ph = 64 * (n % 2), 64 * (n % 2 + 1)
                    nc.tensor.matmul(
                        ps1[:],
                        XTc[n // 16][pl:ph, (n // 2) % 8, 128 * bt:128 * (bt + 1)],
                        RTc[n // 16][pl:ph, (n // 2) % 8, :],
                        start=True, stop=True,
                    )
                    src = ps1[:].rearrange("p (m r) -> p m r", m=M, r=RANK)
                    dst = S[:, :, 8 * n:8 * (n + 1)]
                    if n % 2 == 0:
                        nc.scalar.copy(dst, src)
                    else:
                        nc.vector.tensor_copy(dst, src)

                # ---- butterfly + stage 2, s2 lagged one m-pair behind ----
                tslabs = {}
                for mp in range(33):
                    if mp < 32:
                        psT = psTp.tile([128, 8, 128], BF16, tag="psT", name=f"psT_{bt}_{mp}")
                        for h in range(2):
                            m = 2 * mp + h
                            for g in range(4):
                                nc.tensor.transpose(
                                    psT[:, 4 * h + g, :],
                                    S[:, m, 128 * g:128 * (g + 1)],
                                    ID[:],
                                )
                        Ts = tpool.tile([128, 8, 128], BF16, tag="T", name=f"T_{bt}_{mp}")
                        nc.vector.tensor_copy(Ts[:], psT[:])
                        tslabs[mp] = Ts
                    if mp >= 1:
                        Ts = tslabs.pop(mp - 1)
                        for h in range(2):
                            m = 2 * (mp - 1) + h
                            q = m % 8
                            if q == 0:
                                ps2 = ps2p.tile([128, 8, A], F32, tag="ps2", name=f"ps2_{bt}_{m}")
                            if m == 0:
                                OutSB = opool.tile([128, M, A], BF16, tag="o", name=f"O_{bt}")
                            for g in range(4):
                                nc.tensor.matmul(
                                    ps2[:, q, :],
                                    Ts[:, 4 * h + g, :],
                                    LT[:, m, g, :],
                                    start=(g == 0), stop=(g == 3),
                                )
                            if q == 7:
                                nc.scalar.copy(OutSB[:, m - 7:m + 1, :], ps2[:])
                nc.sync.dma_start(o_ap[bt], OutSB[:])
    nc.compile()
    return nc


def _get_compiled():
    global _compiled
    if _compiled is None:
        _compiled = _build()
    return _compiled


def kernel(x, btt_r, btt_l, bias):
    x = np.asarray(x)
    btt_r = np.asarray(btt_r)
    btt_l = np.asarray(btt_l)
    bias = np.asarray(bias)
    orig_shape = x.shape

    # rt[p, t, j] = btt_r[2t + p//64, p%64, j]
    rt = np.ascontiguousarray(
        btt_r.astype(np.float32).reshape(32, 2, 64, 512).transpose(1, 2, 0, 3)
        .reshape(128, 32, 512)
    ).astype(ml_dtypes.bfloat16)
    # lt[p, m, g, a] = btt_l[m, (16g + p//8)*8 + p%8, a] = l4[m, 16g + p//8, p%8, a]
    l4 = btt_l.astype(np.float32).reshape(M, 4, 16, RANK, A)      # [m, g, nloc, r, a]
    lt = np.ascontiguousarray(l4.transpose(2, 3, 0, 1, 4).reshape(128, M, 4, A)
                              ).astype(ml_dtypes.bfloat16)
    ident = np.eye(128, dtype=ml_dtypes.bfloat16)

    # per-core x shards, transposed: xt[p, t, col] = xs[col, 128t + p]
    xr = x.astype(np.float32).reshape(ROWS, D)
    in_maps = []
    for c in range(N_CORES):
        xs = xr[c * BS:(c + 1) * BS]                               # (BS, D)
        xt = np.ascontiguousarray(
            xs.T.reshape(32, 128, BS).transpose(1, 0, 2)
        ).astype(ml_dtypes.bfloat16)
        in_maps.append({"xt": xt, "rt": rt, "lt": lt, "ident": ident})

    global _last_in_maps
    _last_in_maps = in_maps
    nc = _get_compiled()
    try:
        res = bass_utils.run_bass_kernel_spmd(nc, in_maps, core_ids=list(range(N_CORES)))
    except Exception:
        # transient device hiccups recover on retry
        import time as _time
        _time.sleep(10)
        res = bass_utils.run_bass_kernel_spmd(nc, in_maps, core_ids=list(range(N_CORES)))

    # ---- gather: o (BT, 128, D) bf16 -> rows (BS, D) per core ----
    out = np.empty((ROWS, D), dtype=np.float32)
    for c in range(N_CORES):
        o = np.asarray(res.results[c]["o"], dtype=np.float32)       # (BT, 128, D)
        out[c * BS:(c + 1) * BS] = o.reshape(BS, D)
    out += bias.astype(np.float32)[None, :]
    return out.reshape(*orig_shape[:-1], D)

